# revision 6
# baseline (speedup 1.0000x reference)
"""HGT link predictor on 8 trn2 NeuronCores.

Sharding: nodes split 8 ways per type (2500/core, padded to 2560).
Params replicated. Edges partitioned by destination core, sorted by dst,
packed into 128-edge chunks within 128-dst-node windows. Per layer:
local k_r/v_r projections (relation transforms folded into the weights on
host) -> AllGather -> per-edge dma_gather of k/v (+q) -> logits via
multiply+segmented reduce -> exp -> scatter-add via one-hot matmuls into
PSUM (softmax denominator rides as a 257th..264th column) -> normalize ->
gelu -> output linear -> gated skip + residual + LayerNorm + relu.
"""
import math
import numpy as np

import concourse.bacc as bacc
import concourse.bass as bass
import concourse.mybir as mybir
import concourse.tile as tile
from concourse.bass_utils import run_bass_kernel_spmd
from concourse.library_config import mlp

F32 = mybir.dt.float32
I16 = mybir.dt.int16
AF = mybir.ActivationFunctionType
OP = mybir.AluOpType

T, R, L = 3, 4, 2
H, HEADS, D, FIN, OUT = 256, 8, 32, 128, 128
SRC_T = (0, 1, 1, 1)
DST_T = (1, 0, 1, 2)
LN_EPS = 1e-5
NC = 8
N = 20000
NL = N // NC          # 2500 real local nodes per type
NT = 20               # node tiles of 128
NLP = NT * 128        # 2560 padded local nodes
NWIN = NT             # dst windows of 128 local nodes
GWIN = 1              # windows per gather group
KV_W = 2 * R * H      # 2048: [k0 v0 k1 v1 ...] columns of kv rows


def _block_diag(a):
    """a: [HEADS, D, D] -> [H, H] block diagonal."""
    out = np.zeros((H, H), np.float32)
    for h in range(HEADS):
        out[h * D:(h + 1) * D, h * D:(h + 1) * D] = a[h]
    return out


def _wrap_idx(idx):
    """idx [M] -> [128, M//16] int16 wrapped in 16 partitions, replicated."""
    m = idx.shape[0]
    assert m % 16 == 0
    w = np.zeros((128, m // 16), np.int16)
    w[:16] = idx.astype(np.int16).reshape(m // 16, 16).T
    for rep in range(1, 8):
        w[16 * rep:16 * rep + 16] = w[:16]
    return w


def _preprocess(inputs):
    x = np.asarray(inputs["x"], np.float32)
    edge_index = np.asarray(inputs["edge_index"])
    Win = np.asarray(inputs["Win"], np.float32)
    b_in = np.asarray(inputs["b_in"], np.float32)
    Wk = np.asarray(inputs["Wk"], np.float32); bk = np.asarray(inputs["bk"], np.float32)
    Wq = np.asarray(inputs["Wq"], np.float32); bq = np.asarray(inputs["bq"], np.float32)
    Wv = np.asarray(inputs["Wv"], np.float32); bv = np.asarray(inputs["bv"], np.float32)
    Wa = np.asarray(inputs["Wa"], np.float32); ba = np.asarray(inputs["ba"], np.float32)
    skip = np.asarray(inputs["skip"], np.float32)
    a_rel = np.asarray(inputs["a_rel"], np.float32)
    m_rel = np.asarray(inputs["m_rel"], np.float32)
    p_rel = np.asarray(inputs["p_rel"], np.float32)
    ln_g = np.asarray(inputs["ln_g"], np.float32)
    ln_b = np.asarray(inputs["ln_b"], np.float32)
    Wout = np.asarray(inputs["Wout"], np.float32)
    bout = np.asarray(inputs["bout"], np.float32)

    meta = {}
    # folded weights ------------------------------------------------------
    inv_sqrt_d = 1.0 / math.sqrt(D)
    wkr = np.zeros((L, R, H, H), np.float32); bkr = np.zeros((L, R, H), np.float32)
    wvr = np.zeros((L, R, H, H), np.float32); bvr = np.zeros((L, R, H), np.float32)
    for l in range(L):
        for r in range(R):
            st = SRC_T[r]
            ak = _block_diag(a_rel[l, r] * (p_rel[l, r] * inv_sqrt_d)[:, None, None])
            av = _block_diag(m_rel[l, r])
            wkr[l, r] = Wk[l, st] @ ak; bkr[l, r] = bk[l, st] @ ak
            wvr[l, r] = Wv[l, st] @ av; bvr[l, r] = bv[l, st] @ av
    beta = 1.0 / (1.0 + np.exp(-skip))          # [L, T]
    g = beta / (2.0 - beta)
    wa_eff = Wa * g[:, :, None, None]
    ba_eff = ba * g[:, :, None]
    meta["eps_eff"] = (LN_EPS / (2.0 - beta) ** 2).tolist()

    meta["use_bias"] = dict(
        bin_=bool(np.any(b_in)), bq=bool(np.any(bq)),
        bkr=bool(np.any(bkr)) or bool(np.any(bvr)),
        ba=bool(np.any(ba_eff)), bout=bool(np.any(bout)),
        lng=not np.allclose(ln_g, 1.0), lnb=bool(np.any(ln_b)),
    )

    def bcast(v):
        # [..., F] -> [..., 128, F]: per-feature vectors replicated across partitions
        return np.ascontiguousarray(
            np.broadcast_to(v[..., None, :], v.shape[:-1] + (128, v.shape[-1])))

    # edge partitioning ---------------------------------------------------
    per_core = []
    win_edges = [[] for _ in range(NC)]   # [c][r][w] -> (src_rows, dst_loc)
    kch_need = 1
    for c in range(NC):
        rel = []
        for r in range(R):
            src = edge_index[r, 0].astype(np.int64)
            dst = edge_index[r, 1].astype(np.int64)
            sel = (dst // NL) == c
            s, d = src[sel], dst[sel] - c * NL
            o = np.argsort(d, kind="stable")
            s, d = s[o], d[o]
            wins = []
            for w in range(NWIN):
                m = (d // 128) == w
                sw, dw = s[m], d[m]
                kch_need = max(kch_need, (len(sw) + 127) // 128)
                wins.append((sw, dw))
            rel.append(wins)
        win_edges[c] = rel
    KCH = kch_need
    meta["KCH"] = KCH
    NCHUNK = NWIN * KCH
    NIDX_R = NCHUNK * 128

    for c in range(NC):
        oh = np.zeros((R, NCHUNK, 128, 128), np.float32)
        kv_idx = np.zeros((R, NIDX_R), np.int64)
        qi_idx = np.zeros((R, NIDX_R), np.int64)
        for r in range(R):
            for w in range(NWIN):
                sw, dw = win_edges[c][r][w]
                ne = len(sw)
                base = w * KCH * 128
                # src node n (global) -> kv_full row (n//NL)*NLP + n%NL
                kv_idx[r, base:base + ne] = (sw // NL) * NLP + (sw % NL)
                qi_idx[r, base:base + ne] = dw
                ch = base // 128 + np.arange(ne) // 128
                oh[r, ch, np.arange(ne) % 128, dw - w * 128] = 1.0
        # partition-major one-hot: [R, 128(edge), NCHUNK, 128(col)]
        oh_pm = np.ascontiguousarray(oh.transpose(0, 2, 1, 3))
        xc = np.zeros((T, 128, NLP), np.float32)
        xc[:, :, :NL] = x[:, c * NL:(c + 1) * NL, :].transpose(0, 2, 1)
        per_core.append(dict(
            xT_h=xc,
            oh=oh_pm,
            kv_idx=np.stack([_wrap_idx(kv_idx[r]) for r in range(R)]),
            qi_idx=np.stack([_wrap_idx(qi_idx[r]) for r in range(R)]),
        ))

    shared = dict(
        win=np.ascontiguousarray(Win),                        # [3,128,256]
        wq=np.ascontiguousarray(Wq), wa=np.ascontiguousarray(wa_eff),
        wkr=wkr, wvr=wvr, wout=np.ascontiguousarray(Wout),
        ident=np.eye(128, dtype=np.float32),
        bin_b=bcast(b_in), bq_b=bcast(bq), bkr_b=bcast(bkr), bvr_b=bcast(bvr),
        ba_b=bcast(ba_eff), bout_b=bcast(bout),
        lng_b=bcast(ln_g), lnb_b=bcast(ln_b),
    )
    return shared, per_core, meta


def _build(nc, meta, shapes):
    KCH = meta["KCH"]
    NCHUNK = NWIN * KCH
    GC = GWIN * KCH                      # chunks per gather group
    NGRP = NWIN // GWIN
    ub = meta["use_bias"]
    eps_eff = meta["eps_eff"]

    def din(name):
        return nc.dram_tensor(name, shapes[name], I16 if "idx" in name else F32,
                              kind="ExternalInput").ap()

    xT_h = din("xT_h"); oh_d = din("oh")
    kv_idx_d = din("kv_idx"); qi_idx_d = din("qi_idx")
    win_d = din("win"); wq_d = din("wq"); wa_d = din("wa")
    wkr_d = din("wkr"); wvr_d = din("wvr"); wout_d = din("wout")
    ident_d = din("ident")
    bias_d = {k: din(k) for k in
              ("bin_b", "bq_b", "bkr_b", "bvr_b", "ba_b", "bout_b", "lng_b", "lnb_b")}
    y_d = nc.dram_tensor("y", [T, NLP, OUT], F32, kind="ExternalOutput").ap()

    def bc32(ap2d):
        """[128, k] AP -> [128, k, 32] stride-0 broadcast AP."""
        return bass.AP(tensor=ap2d.tensor, offset=ap2d.offset,
                       ap=list(ap2d.ap) + [[0, D]])

    with tile.TileContext(nc) as tc:
        with (
            tc.tile_pool(name="persist", bufs=1) as pp,
            tc.tile_pool(name="wpool", bufs=3) as wp,
            tc.tile_pool(name="stage", bufs=4) as stg,
            tc.tile_pool(name="edge", bufs=2) as ep,
            tc.tile_pool(name="edge1", bufs=1) as ep1,
            tc.tile_pool(name="small", bufs=4) as sp,
            tc.tile_pool(name="psA", bufs=2, space="PSUM") as psA,
            tc.tile_pool(name="psB", bufs=2, space="PSUM") as psB,
            tc.tile_pool(name="psC", bufs=2, space="PSUM") as psC,
            tc.tile_pool(name="dram", bufs=1, space="DRAM") as dp,
        ):
            nc.gpsimd.load_library(mlp)

            ident = pp.tile([128, 128], F32, tag="ident")
            nc.sync.dma_start(ident[:], ident_d)
            h = pp.tile([128, T, NT, H], F32, tag="h")
            agg1 = pp.tile([128, NT, H], F32, tag="agg1")

            kv_loc = dp.tile([NLP, KV_W], F32)
            kv_full = dp.tile([NC * NLP, KV_W], F32)
            q_dram = dp.tile([T, NLP, H], F32)

            def load_w(src_ap):
                """[256, M] dram -> [128, 2, M] sbuf tile."""
                m = src_ap.shape[-1]
                t_ = wp.tile([128, 2, m], F32, tag="w")
                nc.sync.dma_start(t_[:], src_ap.rearrange("(kt kp) m -> kp kt m", kp=128))
                return t_

            def load_bias(src_ap):
                t_ = wp.tile([128, H], F32, tag="bias")
                nc.sync.dma_start(t_[:], src_ap)
                return t_

            # ---- input projection: h[t] = relu(xT^T @ Win + b) ----
            for t in range(T):
                w_in = wp.tile([128, H], F32, tag="w")
                nc.sync.dma_start(w_in[:], win_d[t])
                bt = load_bias(bias_d["bin_b"][t]) if ub["bin_"] else None
                for nt in range(NT):
                    xt = wp.tile([128, 128], F32, tag="xt")
                    nc.sync.dma_start(xt[:], xT_h[t, :, nt * 128:(nt + 1) * 128])
                    ps = psA.tile([128, H], F32)
                    nc.tensor.matmul(ps[:], xt[:],
                                     w_in[:], start=True, stop=True)
                    if bt is not None:
                        nc.vector.tensor_add(ps[:], ps[:], bt[:])
                    nc.scalar.activation(h[:, t, nt, :], ps[:], AF.Relu)

            hT = pp.tile([128, 2, NT, 128], F32, tag="hT")

            def transpose_to(dst3, src2, nt_label):
                """src2 [128, 256] sbuf -> dst3 [128, 2, 128] (feature-major)."""
                for ft in range(2):
                    tp = psB.tile([128, 128], F32)
                    nc.tensor.transpose(tp[:], src2[:, ft * 128:(ft + 1) * 128], ident[:])
                    eng = nc.vector if (nt_label + ft) % 2 else nc.scalar
                    eng.tensor_copy(dst3[:, ft, :], tp[:]) if eng is nc.vector \
                        else nc.scalar.copy(dst3[:, ft, :], tp[:])

            def proj_to_dram(wtile, btile, dst_rows, col0, ncols):
                """out rows = hT^T @ w (+bias) -> dram[dst_rows, col0:col0+ncols]"""
                for nt in range(NT):
                    ps = psA.tile([128, ncols], F32)
                    for kt in range(2):
                        nc.tensor.matmul(ps[:], hT[:, kt, nt, :], wtile[:, kt, :],
                                         start=(kt == 0), stop=(kt == 1))
                    st = stg.tile([128, H], F32, tag="projout")
                    if btile is not None:
                        nc.vector.tensor_add(st[:, :ncols], ps[:], btile[:, :ncols])
                    else:
                        nc.scalar.copy(st[:, :ncols], ps[:])
                    nc.sync.dma_start(
                        dst_rows[nt * 128:(nt + 1) * 128, col0:col0 + ncols],
                        st[:, :ncols])

            for l in range(L):
                # ---- per-type transposes + projections ----
                for t in range(T):
                    for nt in range(NT):
                        transpose_to(hT[:, :, nt, :], h[:, t, nt, :], nt)
                    wq_t = load_w(wq_d[l, t])
                    bq_t = load_bias(bias_d["bq_b"][l, t]) if ub["bq"] else None
                    proj_to_dram(wq_t, bq_t, q_dram[t], 0, H)
                    for r in range(R):
                        if SRC_T[r] != t:
                            continue
                        wk_t = load_w(wkr_d[l, r])
                        bk_t = load_bias(bias_d["bkr_b"][l, r]) if ub["bkr"] else None
                        proj_to_dram(wk_t, bk_t, kv_loc[:], (2 * r) * H, H)
                        wv_t = load_w(wvr_d[l, r])
                        bv_t = load_bias(bias_d["bvr_b"][l, r]) if ub["bkr"] else None
                        proj_to_dram(wv_t, bv_t, kv_loc[:], (2 * r + 1) * H, H)

                nc.gpsimd.collective_compute(
                    "AllGather", OP.bypass,
                    replica_groups=[list(range(NC))],
                    ins=[kv_loc[:].opt()], outs=[kv_full[:].opt()],
                )

                # ---- edge phase;  r order: 0 (t1 agg), 1 (t0), 2 (t1+post), 3 (t2) ----
                s1 = sp.tile([128, NT], F32, tag="s1")
                s2 = sp.tile([128, NT], F32, tag="s2")
                sqs = stg.tile([128, H], F32, tag="sqs")

                def post_edge_window(t, w, agg_norm, wa_t, ba_t):
                    gt = stg.tile([128, H], F32, tag="gelu")
                    nc.scalar.activation(gt[:], agg_norm, AF.Gelu)
                    gT = stg.tile([128, 2, 128], F32, tag="gT")
                    transpose_to(gT[:], gt[:], w)
                    po = psA.tile([128, H], F32)
                    for kt in range(2):
                        nc.tensor.matmul(po[:], gT[:, kt, :], wa_t[:, kt, :],
                                         start=(kt == 0), stop=(kt == 1))
                    if ba_t is not None:
                        nc.vector.tensor_add(po[:], po[:], ba_t[:])
                    # h_pre = o + h (in place), s1 = row sums
                    nc.vector.scalar_tensor_tensor(
                        h[:, t, w, :], po[:], 1.0, h[:, t, w, :],
                        OP.mult, OP.add, accum_out=s1[:, w:w + 1])
                    nc.scalar.activation(sqs[:], h[:, t, w, :], AF.Square,
                                         accum_out=s2[:, w:w + 1])

                def finish_type(t, l):
                    mu = sp.tile([128, NT], F32, tag="mu")
                    inv = sp.tile([128, NT], F32, tag="inv")
                    nmi = sp.tile([128, NT], F32, tag="nmi")
                    nc.vector.tensor_scalar_mul(mu[:], s1[:], 1.0 / H)
                    nc.vector.tensor_scalar_mul(inv[:], s2[:], 1.0 / H)  # mean sq
                    musq = sp.tile([128, NT], F32, tag="musq")
                    nc.vector.tensor_mul(musq[:], mu[:], mu[:])
                    nc.vector.scalar_tensor_tensor(
                        inv[:], inv[:], float(eps_eff[l][t]), musq[:],
                        OP.add, OP.subtract)              # var + eps
                    nc.scalar.activation(inv[:], inv[:], AF.Sqrt)
                    nc.vector.reciprocal(inv[:], inv[:])
                    nc.vector.scalar_tensor_tensor(
                        nmi[:], mu[:], -1.0, inv[:], OP.mult, OP.mult)
                    if ub["lng"] or ub["lnb"]:
                        lng_t = load_bias(bias_d["lng_b"][l, t])
                        lnb_t = load_bias(bias_d["lnb_b"][l, t])
                        for w in range(NT):
                            nc.scalar.activation(
                                h[:, t, w, :], h[:, t, w, :], AF.Identity,
                                bias=nmi[:, w:w + 1], scale=inv[:, w:w + 1])
                            nc.vector.tensor_mul(h[:, t, w, :], h[:, t, w, :], lng_t[:])
                            nc.vector.tensor_add(h[:, t, w, :], h[:, t, w, :], lnb_t[:])
                            nc.scalar.activation(h[:, t, w, :], h[:, t, w, :], AF.Relu)
                    else:
                        for w in range(NT):
                            nc.scalar.activation(
                                h[:, t, w, :], h[:, t, w, :], AF.Relu,
                                bias=nmi[:, w:w + 1], scale=inv[:, w:w + 1])

                for r in (0, 1, 2, 3):
                    dt = DST_T[r]
                    wa_t = ba_t = None
                    if r != 0:
                        wa_t = load_w(wa_d[l, dt])
                        ba_t = load_bias(bias_d["ba_b"][l, dt]) if ub["ba"] else None
                    kvi = sp.tile([128, NIDX_R16(KCH)], I16, tag="kvi")
                    qii = sp.tile([128, NIDX_R16(KCH)], I16, tag="qii")
                    nc.gpsimd.dma_start(kvi[:], kv_idx_d[r])
                    nc.gpsimd.dma_start(qii[:], qi_idx_d[r])
                    for gidx in range(NGRP):
                        ni = GC * 128
                        kvg = ep.tile([128, GC, 2 * H], F32, tag="kvg")
                        qig = ep.tile([128, GC, H], F32, tag="qig")
                        nc.gpsimd.dma_gather(
                            kvg[:], kv_full[:, (2 * r) * H:(2 * r + 2) * H],
                            kvi[:, gidx * (ni // 16):(gidx + 1) * (ni // 16)],
                            ni, ni, 2 * H, elem_step=KV_W)
                        nc.gpsimd.dma_gather(
                            qig[:], q_dram[dt],
                            qii[:, gidx * (ni // 16):(gidx + 1) * (ni // 16)],
                            ni, ni, H)
                        ohg = ep.tile([128, GC, 128], F32, tag="ohg")
                        nc.sync.dma_start(ohg[:], oh_d[r, :, gidx * GC:(gidx + 1) * GC, :])
                        msg = ep1.tile([128, GC, H + HEADS], F32, tag="msg")
                        lg = sp.tile([128, GC, HEADS], F32, tag="lg")
                        nc.vector.tensor_mul(msg[:, :, 0:H], qig[:], kvg[:, :, 0:H])
                        nc.vector.tensor_reduce(
                            lg[:], msg[:, :, 0:H].rearrange("p g (hh dd) -> p g hh dd", dd=D),
                            mybir.AxisListType.X, OP.add)
                        nc.scalar.activation(msg[:, :, H:H + HEADS], lg[:], AF.Exp)
                        nc.vector.tensor_mul(
                            msg[:, :, 0:H].rearrange("p g (hh dd) -> p g hh dd", dd=D),
                            kvg[:, :, H:2 * H].rearrange("p g (hh dd) -> p g hh dd", dd=D),
                            bc32(msg[:, :, H:H + HEADS]))
                        for wi in range(GWIN):
                            w = gidx * GWIN + wi
                            pw = psC.tile([128, H + HEADS], F32)
                            for kc in range(KCH):
                                nc.tensor.matmul(
                                    pw[:], ohg[:, wi * KCH + kc, :],
                                    msg[:, wi * KCH + kc, :],
                                    start=(kc == 0), stop=(kc == KCH - 1))
                            rec = sp.tile([128, HEADS], F32, tag="rec")
                            # +1e-30: degree-0 dst nodes have sum 0; keep 0*recip = 0
                            nc.vector.tensor_scalar_add(rec[:], pw[:, H:H + HEADS], 1e-30)
                            nc.vector.reciprocal(rec[:], rec[:])
                            if r == 0:
                                nc.vector.tensor_mul(
                                    agg1[:, w, :].rearrange("p (hh dd) -> p hh dd", dd=D),
                                    pw[:, 0:H].rearrange("p (hh dd) -> p hh dd", dd=D),
                                    bc32(rec[:]))
                            else:
                                an = stg.tile([128, H], F32, tag="aggn")
                                nc.vector.tensor_mul(
                                    an[:].rearrange("p (hh dd) -> p hh dd", dd=D),
                                    pw[:, 0:H].rearrange("p (hh dd) -> p hh dd", dd=D),
                                    bc32(rec[:]))
                                if r == 2:
                                    nc.vector.tensor_add(an[:], an[:], agg1[:, w, :])
                                post_edge_window(dt, w, an[:], wa_t, ba_t)
                    if r != 0:
                        finish_type(dt, l)

            # ---- output projection ----
            wo = load_w(wout_d)
            bo = load_bias(bias_d["bout_b"]) if ub["bout"] else None
            for t in range(T):
                for nt in range(NT):
                    transpose_to(hT[:, :, nt, :], h[:, t, nt, :], nt)
                    ps = psA.tile([128, OUT], F32)
                    for kt in range(2):
                        nc.tensor.matmul(ps[:], hT[:, kt, nt, :], wo[:, kt, :OUT],
                                         start=(kt == 0), stop=(kt == 1))
                    st = stg.tile([128, OUT], F32, tag="yout")
                    if bo is not None:
                        nc.vector.tensor_add(st[:], ps[:], bo[:, :OUT])
                    else:
                        nc.scalar.copy(st[:], ps[:])
                    nc.sync.dma_start(y_d[t, nt * 128:(nt + 1) * 128, :], st[:])
    nc.compile()
    return nc


def NIDX_R16(KCH):
    return NWIN * KCH * 128 // 16


def kernel(**inputs):
    shared, per_core, meta = _preprocess(inputs)
    shapes = {k: list(v.shape) for k, v in {**shared, **per_core[0]}.items()}
    nc = bacc.Bacc("TRN2", target_bir_lowering=False, debug=False, num_devices=NC)
    nc = _build(nc, meta, shapes)
    in_maps = [{**shared, **per_core[c]} for c in range(NC)]
    res = run_bass_kernel_spmd(nc, in_maps, core_ids=list(range(NC)))
    y = np.concatenate([res.results[c]["y"][:, :NL, :] for c in range(NC)], axis=1)
    return y.astype(np.float32)


if __name__ == "__main__":
    import reference
    inputs = {k: np.asarray(v) for k, v in reference.setup_inputs().items()}
    out = kernel(**inputs)
    exp = np.asarray(reference.reference(**inputs))
    err = np.abs(out - exp).max() / np.abs(exp).max()
    print("Relative error:", err)


# revision 7
# speedup vs baseline: 1.4341x; 1.4341x over previous
"""HGT link predictor on 8 trn2 NeuronCores.

Sharding: nodes split 8 ways per type (2500/core, padded to 2560).
Params replicated. Edges partitioned by destination core, sorted by dst,
packed into 128-edge chunks within 128-dst-node windows. Per layer:
local k_r/v_r projections (relation transforms folded into the weights on
host) -> AllGather -> per-edge dma_gather of k/v (+q) -> logits via
multiply+segmented reduce -> exp -> scatter-add via one-hot matmuls into
PSUM (softmax denominator rides as a 257th..264th column) -> normalize ->
gelu -> output linear -> gated skip + residual + LayerNorm + relu.
"""
import math
import numpy as np

import concourse.bacc as bacc
import concourse.bass as bass
import concourse.mybir as mybir
import concourse.tile as tile
from concourse.bass_utils import run_bass_kernel_spmd
from concourse.library_config import mlp

F32 = mybir.dt.float32
I16 = mybir.dt.int16
AF = mybir.ActivationFunctionType
OP = mybir.AluOpType

T, R, L = 3, 4, 2
H, HEADS, D, FIN, OUT = 256, 8, 32, 128, 128
SRC_T = (0, 1, 1, 1)
DST_T = (1, 0, 1, 2)
LN_EPS = 1e-5
NC = 8
N = 20000
NL = N // NC          # 2500 real local nodes per type
NT = 20               # node tiles of 128
NLP = NT * 128        # 2560 padded local nodes
NWIN = NT             # dst windows of 128 local nodes
GWIN = 2              # windows per gather group
KV_W = 2 * R * H      # 2048: [k0 v0 k1 v1 ...] columns of kv rows


def _block_diag(a):
    """a: [HEADS, D, D] -> [H, H] block diagonal."""
    out = np.zeros((H, H), np.float32)
    for h in range(HEADS):
        out[h * D:(h + 1) * D, h * D:(h + 1) * D] = a[h]
    return out


def _wrap_idx(idx):
    """idx [M] -> [128, M//16] int16 wrapped in 16 partitions, replicated."""
    m = idx.shape[0]
    assert m % 16 == 0
    w = np.zeros((128, m // 16), np.int16)
    w[:16] = idx.astype(np.int16).reshape(m // 16, 16).T
    for rep in range(1, 8):
        w[16 * rep:16 * rep + 16] = w[:16]
    return w


def _preprocess(inputs):
    x = np.asarray(inputs["x"], np.float32)
    edge_index = np.asarray(inputs["edge_index"])
    Win = np.asarray(inputs["Win"], np.float32)
    b_in = np.asarray(inputs["b_in"], np.float32)
    Wk = np.asarray(inputs["Wk"], np.float32); bk = np.asarray(inputs["bk"], np.float32)
    Wq = np.asarray(inputs["Wq"], np.float32); bq = np.asarray(inputs["bq"], np.float32)
    Wv = np.asarray(inputs["Wv"], np.float32); bv = np.asarray(inputs["bv"], np.float32)
    Wa = np.asarray(inputs["Wa"], np.float32); ba = np.asarray(inputs["ba"], np.float32)
    skip = np.asarray(inputs["skip"], np.float32)
    a_rel = np.asarray(inputs["a_rel"], np.float32)
    m_rel = np.asarray(inputs["m_rel"], np.float32)
    p_rel = np.asarray(inputs["p_rel"], np.float32)
    ln_g = np.asarray(inputs["ln_g"], np.float32)
    ln_b = np.asarray(inputs["ln_b"], np.float32)
    Wout = np.asarray(inputs["Wout"], np.float32)
    bout = np.asarray(inputs["bout"], np.float32)

    meta = {}
    # folded weights ------------------------------------------------------
    inv_sqrt_d = 1.0 / math.sqrt(D)
    wkr = np.zeros((L, R, H, H), np.float32); bkr = np.zeros((L, R, H), np.float32)
    wvr = np.zeros((L, R, H, H), np.float32); bvr = np.zeros((L, R, H), np.float32)
    for l in range(L):
        for r in range(R):
            st = SRC_T[r]
            ak = _block_diag(a_rel[l, r] * (p_rel[l, r] * inv_sqrt_d)[:, None, None])
            av = _block_diag(m_rel[l, r])
            wkr[l, r] = Wk[l, st] @ ak; bkr[l, r] = bk[l, st] @ ak
            wvr[l, r] = Wv[l, st] @ av; bvr[l, r] = bv[l, st] @ av
    beta = 1.0 / (1.0 + np.exp(-skip))          # [L, T]
    g = beta / (2.0 - beta)
    wa_eff = Wa * g[:, :, None, None]
    ba_eff = ba * g[:, :, None]
    meta["eps_eff"] = (LN_EPS / (2.0 - beta) ** 2).tolist()

    meta["use_bias"] = dict(
        bin_=bool(np.any(b_in)), bq=bool(np.any(bq)),
        bkr=bool(np.any(bkr)) or bool(np.any(bvr)),
        ba=bool(np.any(ba_eff)), bout=bool(np.any(bout)),
        lng=not np.allclose(ln_g, 1.0), lnb=bool(np.any(ln_b)),
    )

    def bcast(v):
        # [..., F] -> [..., 128, F]: per-feature vectors replicated across partitions
        return np.ascontiguousarray(
            np.broadcast_to(v[..., None, :], v.shape[:-1] + (128, v.shape[-1])))

    # edge partitioning ---------------------------------------------------
    per_core = []
    win_edges = [[] for _ in range(NC)]   # [c][r][w] -> (src_rows, dst_loc)
    kch_need = 1
    for c in range(NC):
        rel = []
        for r in range(R):
            src = edge_index[r, 0].astype(np.int64)
            dst = edge_index[r, 1].astype(np.int64)
            sel = (dst // NL) == c
            s, d = src[sel], dst[sel] - c * NL
            o = np.argsort(d, kind="stable")
            s, d = s[o], d[o]
            wins = []
            for w in range(NWIN):
                m = (d // 128) == w
                sw, dw = s[m], d[m]
                kch_need = max(kch_need, (len(sw) + 127) // 128)
                wins.append((sw, dw))
            rel.append(wins)
        win_edges[c] = rel
    KCH = kch_need
    meta["KCH"] = KCH
    NCHUNK = NWIN * KCH
    NIDX_R = NCHUNK * 128

    for c in range(NC):
        oh = np.zeros((R, NCHUNK, 128, 128), np.float32)
        kv_idx = np.zeros((R, NIDX_R), np.int64)
        qi_idx = np.zeros((R, NIDX_R), np.int64)
        for r in range(R):
            for w in range(NWIN):
                sw, dw = win_edges[c][r][w]
                ne = len(sw)
                base = w * KCH * 128
                # src node n (global) -> kv_full row (n//NL)*NLP + n%NL
                kv_idx[r, base:base + ne] = (sw // NL) * NLP + (sw % NL)
                qi_idx[r, base:base + ne] = dw
                ch = base // 128 + np.arange(ne) // 128
                oh[r, ch, np.arange(ne) % 128, dw - w * 128] = 1.0
        # partition-major one-hot: [R, 128(edge), NCHUNK, 128(col)]
        oh_pm = np.ascontiguousarray(oh.transpose(0, 2, 1, 3))
        xc = np.zeros((T, 128, NLP), np.float32)
        xc[:, :, :NL] = x[:, c * NL:(c + 1) * NL, :].transpose(0, 2, 1)
        per_core.append(dict(
            xT_h=xc,
            oh=oh_pm,
            kv_idx=np.stack([_wrap_idx(kv_idx[r]) for r in range(R)]),
            qi_idx=np.stack([_wrap_idx(qi_idx[r]) for r in range(R)]),
        ))

    shared = dict(
        win=np.ascontiguousarray(Win),                        # [3,128,256]
        wq=np.ascontiguousarray(Wq), wa=np.ascontiguousarray(wa_eff),
        wkr=wkr, wvr=wvr, wout=np.ascontiguousarray(Wout),
        ident=np.eye(128, dtype=np.float32),
        bin_b=bcast(b_in), bq_b=bcast(bq), bkr_b=bcast(bkr), bvr_b=bcast(bvr),
        ba_b=bcast(ba_eff), bout_b=bcast(bout),
        lng_b=bcast(ln_g), lnb_b=bcast(ln_b),
    )
    return shared, per_core, meta


def _build(nc, meta, shapes):
    KCH = meta["KCH"]
    NCHUNK = NWIN * KCH
    GC = GWIN * KCH                      # chunks per gather group
    NGRP = NWIN // GWIN
    ub = meta["use_bias"]
    eps_eff = meta["eps_eff"]

    def din(name):
        return nc.dram_tensor(name, shapes[name], I16 if "idx" in name else F32,
                              kind="ExternalInput").ap()

    xT_h = din("xT_h"); oh_d = din("oh")
    kv_idx_d = din("kv_idx"); qi_idx_d = din("qi_idx")
    win_d = din("win"); wq_d = din("wq"); wa_d = din("wa")
    wkr_d = din("wkr"); wvr_d = din("wvr"); wout_d = din("wout")
    ident_d = din("ident")
    bias_d = {k: din(k) for k in
              ("bin_b", "bq_b", "bkr_b", "bvr_b", "ba_b", "bout_b", "lng_b", "lnb_b")}
    y_d = nc.dram_tensor("y", [T, NLP, OUT], F32, kind="ExternalOutput").ap()

    def bc32(ap2d):
        """[128, k] AP -> [128, k, 32] stride-0 broadcast AP."""
        return bass.AP(tensor=ap2d.tensor, offset=ap2d.offset,
                       ap=list(ap2d.ap) + [[0, D]])

    with tile.TileContext(nc) as tc:
        with (
            tc.tile_pool(name="persist", bufs=1) as pp,
            tc.tile_pool(name="wpool", bufs=3) as wp,
            tc.tile_pool(name="stage", bufs=4) as stg,
            tc.tile_pool(name="edge", bufs=2) as ep,
            tc.tile_pool(name="edge1", bufs=1) as ep1,
            tc.tile_pool(name="small", bufs=4) as sp,
            tc.tile_pool(name="psA", bufs=2, space="PSUM") as psA,
            tc.tile_pool(name="psB", bufs=2, space="PSUM") as psB,
            tc.tile_pool(name="psC", bufs=2, space="PSUM") as psC,
            tc.tile_pool(name="dram", bufs=1, space="DRAM") as dp,
        ):
            nc.gpsimd.load_library(mlp)

            ident = pp.tile([128, 128], F32, tag="ident")
            nc.sync.dma_start(ident[:], ident_d)
            h = pp.tile([128, T, NT, H], F32, tag="h")
            agg1 = pp.tile([128, NT, H], F32, tag="agg1")

            kv_loc = dp.tile([NLP, KV_W], F32)
            kv_full = dp.tile([NC * NLP, KV_W], F32)
            q_dram = dp.tile([T, NLP, H], F32)

            def load_w(src_ap):
                """[256, M] dram -> [128, 2, M] sbuf tile."""
                m = src_ap.shape[-1]
                t_ = wp.tile([128, 2, m], F32, tag="w")
                nc.sync.dma_start(t_[:], src_ap.rearrange("(kt kp) m -> kp kt m", kp=128))
                return t_

            def load_bias(src_ap):
                t_ = wp.tile([128, H], F32, tag="bias")
                nc.sync.dma_start(t_[:], src_ap)
                return t_

            # ---- input projection: h[t] = relu(xT^T @ Win + b) ----
            for t in range(T):
                w_in = wp.tile([128, H], F32, tag="w")
                nc.sync.dma_start(w_in[:], win_d[t])
                bt = load_bias(bias_d["bin_b"][t]) if ub["bin_"] else None
                for nt in range(NT):
                    xt = wp.tile([128, 128], F32, tag="xt")
                    nc.sync.dma_start(xt[:], xT_h[t, :, nt * 128:(nt + 1) * 128])
                    ps = psA.tile([128, H], F32)
                    nc.tensor.matmul(ps[:], xt[:],
                                     w_in[:], start=True, stop=True)
                    if bt is not None:
                        nc.vector.tensor_add(ps[:], ps[:], bt[:])
                    nc.scalar.activation(h[:, t, nt, :], ps[:], AF.Relu)

            hT = pp.tile([128, 2, NT, 128], F32, tag="hT")

            def transpose_to(dst3, src2, nt_label):
                """src2 [128, 256] sbuf -> dst3 [128, 2, 128] (feature-major)."""
                for ft in range(2):
                    tp = psB.tile([128, 128], F32)
                    nc.tensor.transpose(tp[:], src2[:, ft * 128:(ft + 1) * 128], ident[:])
                    eng = nc.vector if (nt_label + ft) % 2 else nc.scalar
                    eng.tensor_copy(dst3[:, ft, :], tp[:]) if eng is nc.vector \
                        else nc.scalar.copy(dst3[:, ft, :], tp[:])

            def proj_to_dram(wtile, btile, dst_rows, col0, ncols):
                """out rows = hT^T @ w (+bias) -> dram[dst_rows, col0:col0+ncols]"""
                for nt in range(NT):
                    ps = psA.tile([128, ncols], F32)
                    for kt in range(2):
                        nc.tensor.matmul(ps[:], hT[:, kt, nt, :], wtile[:, kt, :],
                                         start=(kt == 0), stop=(kt == 1))
                    st = stg.tile([128, H], F32, tag="projout")
                    if btile is not None:
                        nc.vector.tensor_add(st[:, :ncols], ps[:], btile[:, :ncols])
                    else:
                        nc.scalar.copy(st[:, :ncols], ps[:])
                    nc.sync.dma_start(
                        dst_rows[nt * 128:(nt + 1) * 128, col0:col0 + ncols],
                        st[:, :ncols])

            for l in range(L):
                # ---- per-type transposes + projections ----
                for t in range(T):
                    for nt in range(NT):
                        transpose_to(hT[:, :, nt, :], h[:, t, nt, :], nt)
                    wq_t = load_w(wq_d[l, t])
                    bq_t = load_bias(bias_d["bq_b"][l, t]) if ub["bq"] else None
                    proj_to_dram(wq_t, bq_t, q_dram[t], 0, H)
                    for r in range(R):
                        if SRC_T[r] != t:
                            continue
                        wk_t = load_w(wkr_d[l, r])
                        bk_t = load_bias(bias_d["bkr_b"][l, r]) if ub["bkr"] else None
                        proj_to_dram(wk_t, bk_t, kv_loc[:], (2 * r) * H, H)
                        wv_t = load_w(wvr_d[l, r])
                        bv_t = load_bias(bias_d["bvr_b"][l, r]) if ub["bkr"] else None
                        proj_to_dram(wv_t, bv_t, kv_loc[:], (2 * r + 1) * H, H)

                nc.gpsimd.collective_compute(
                    "AllGather", OP.bypass,
                    replica_groups=[list(range(NC))],
                    ins=[kv_loc[:].opt()], outs=[kv_full[:].opt()],
                )

                # ---- edge phase;  r order: 0 (t1 agg), 1 (t0), 2 (t1+post), 3 (t2) ----
                s1 = sp.tile([128, NT], F32, tag="s1")
                s2 = sp.tile([128, NT], F32, tag="s2")
                sqs = stg.tile([128, H], F32, tag="sqs")

                def post_edge_window(t, w, agg_norm, wa_t, ba_t):
                    gt = stg.tile([128, H], F32, tag="gelu")
                    nc.scalar.activation(gt[:], agg_norm, AF.Gelu)
                    gT = stg.tile([128, 2, 128], F32, tag="gT")
                    transpose_to(gT[:], gt[:], w)
                    po = psA.tile([128, H], F32)
                    for kt in range(2):
                        nc.tensor.matmul(po[:], gT[:, kt, :], wa_t[:, kt, :],
                                         start=(kt == 0), stop=(kt == 1))
                    if ba_t is not None:
                        nc.vector.tensor_add(po[:], po[:], ba_t[:])
                    # h_pre = o + h (in place), s1 = row sums
                    nc.vector.scalar_tensor_tensor(
                        h[:, t, w, :], po[:], 1.0, h[:, t, w, :],
                        OP.mult, OP.add, accum_out=s1[:, w:w + 1])
                    nc.scalar.activation(sqs[:], h[:, t, w, :], AF.Square,
                                         accum_out=s2[:, w:w + 1])

                def finish_type(t, l):
                    mu = sp.tile([128, NT], F32, tag="mu")
                    inv = sp.tile([128, NT], F32, tag="inv")
                    nmi = sp.tile([128, NT], F32, tag="nmi")
                    nc.vector.tensor_scalar_mul(mu[:], s1[:], 1.0 / H)
                    nc.vector.tensor_scalar_mul(inv[:], s2[:], 1.0 / H)  # mean sq
                    musq = sp.tile([128, NT], F32, tag="musq")
                    nc.vector.tensor_mul(musq[:], mu[:], mu[:])
                    nc.vector.scalar_tensor_tensor(
                        inv[:], inv[:], float(eps_eff[l][t]), musq[:],
                        OP.add, OP.subtract)              # var + eps
                    nc.scalar.activation(inv[:], inv[:], AF.Sqrt)
                    nc.vector.reciprocal(inv[:], inv[:])
                    nc.vector.scalar_tensor_tensor(
                        nmi[:], mu[:], -1.0, inv[:], OP.mult, OP.mult)
                    if ub["lng"] or ub["lnb"]:
                        lng_t = load_bias(bias_d["lng_b"][l, t])
                        lnb_t = load_bias(bias_d["lnb_b"][l, t])
                        for w in range(NT):
                            nc.scalar.activation(
                                h[:, t, w, :], h[:, t, w, :], AF.Identity,
                                bias=nmi[:, w:w + 1], scale=inv[:, w:w + 1])
                            nc.vector.tensor_mul(h[:, t, w, :], h[:, t, w, :], lng_t[:])
                            nc.vector.tensor_add(h[:, t, w, :], h[:, t, w, :], lnb_t[:])
                            nc.scalar.activation(h[:, t, w, :], h[:, t, w, :], AF.Relu)
                    else:
                        for w in range(NT):
                            nc.scalar.activation(
                                h[:, t, w, :], h[:, t, w, :], AF.Relu,
                                bias=nmi[:, w:w + 1], scale=inv[:, w:w + 1])

                for r in (0, 1, 2, 3):
                    dt = DST_T[r]
                    wa_t = ba_t = None
                    if r != 0:
                        wa_t = load_w(wa_d[l, dt])
                        ba_t = load_bias(bias_d["ba_b"][l, dt]) if ub["ba"] else None
                    kvi = sp.tile([128, NIDX_R16(KCH)], I16, tag="kvi")
                    qii = sp.tile([128, NIDX_R16(KCH)], I16, tag="qii")
                    nc.gpsimd.dma_start(kvi[:], kv_idx_d[r])
                    nc.gpsimd.dma_start(qii[:], qi_idx_d[r])
                    for gidx in range(NGRP):
                        ni = GC * 128
                        kvg = ep.tile([128, GC, 2 * H], F32, tag="kvg")
                        qig = ep.tile([128, GC, H], F32, tag="qig")
                        nc.gpsimd.dma_gather(
                            kvg[:], kv_full[:, (2 * r) * H:(2 * r + 2) * H],
                            kvi[:, gidx * (ni // 16):(gidx + 1) * (ni // 16)],
                            ni, ni, 2 * H, elem_step=KV_W)
                        nc.gpsimd.dma_gather(
                            qig[:], q_dram[dt],
                            qii[:, gidx * (ni // 16):(gidx + 1) * (ni // 16)],
                            ni, ni, H)
                        ohg = ep.tile([128, GC, 128], F32, tag="ohg")
                        nc.sync.dma_start(ohg[:], oh_d[r, :, gidx * GC:(gidx + 1) * GC, :])
                        msg = ep1.tile([128, GC, H + HEADS], F32, tag="msg")
                        lg = sp.tile([128, GC, HEADS], F32, tag="lg")
                        nc.vector.tensor_mul(msg[:, :, 0:H], qig[:], kvg[:, :, 0:H])
                        nc.vector.tensor_reduce(
                            lg[:], msg[:, :, 0:H].rearrange("p g (hh dd) -> p g hh dd", dd=D),
                            mybir.AxisListType.X, OP.add)
                        nc.scalar.activation(msg[:, :, H:H + HEADS], lg[:], AF.Exp)
                        nc.vector.tensor_mul(
                            msg[:, :, 0:H].rearrange("p g (hh dd) -> p g hh dd", dd=D),
                            kvg[:, :, H:2 * H].rearrange("p g (hh dd) -> p g hh dd", dd=D),
                            bc32(msg[:, :, H:H + HEADS]))
                        for wi in range(GWIN):
                            w = gidx * GWIN + wi
                            pw = psC.tile([128, H + HEADS], F32)
                            for kc in range(KCH):
                                nc.tensor.matmul(
                                    pw[:], ohg[:, wi * KCH + kc, :],
                                    msg[:, wi * KCH + kc, :],
                                    start=(kc == 0), stop=(kc == KCH - 1))
                            rec = sp.tile([128, HEADS], F32, tag="rec")
                            # +1e-30: degree-0 dst nodes have sum 0; keep 0*recip = 0
                            nc.vector.tensor_scalar_add(rec[:], pw[:, H:H + HEADS], 1e-30)
                            nc.vector.reciprocal(rec[:], rec[:])
                            if r == 0:
                                nc.vector.tensor_mul(
                                    agg1[:, w, :].rearrange("p (hh dd) -> p hh dd", dd=D),
                                    pw[:, 0:H].rearrange("p (hh dd) -> p hh dd", dd=D),
                                    bc32(rec[:]))
                            else:
                                an = stg.tile([128, H], F32, tag="aggn")
                                nc.vector.tensor_mul(
                                    an[:].rearrange("p (hh dd) -> p hh dd", dd=D),
                                    pw[:, 0:H].rearrange("p (hh dd) -> p hh dd", dd=D),
                                    bc32(rec[:]))
                                if r == 2:
                                    nc.vector.tensor_add(an[:], an[:], agg1[:, w, :])
                                post_edge_window(dt, w, an[:], wa_t, ba_t)
                    if r != 0:
                        finish_type(dt, l)

            # ---- output projection ----
            wo = load_w(wout_d)
            bo = load_bias(bias_d["bout_b"]) if ub["bout"] else None
            for t in range(T):
                for nt in range(NT):
                    transpose_to(hT[:, :, nt, :], h[:, t, nt, :], nt)
                    ps = psA.tile([128, OUT], F32)
                    for kt in range(2):
                        nc.tensor.matmul(ps[:], hT[:, kt, nt, :], wo[:, kt, :OUT],
                                         start=(kt == 0), stop=(kt == 1))
                    st = stg.tile([128, OUT], F32, tag="yout")
                    if bo is not None:
                        nc.vector.tensor_add(st[:], ps[:], bo[:, :OUT])
                    else:
                        nc.scalar.copy(st[:], ps[:])
                    nc.sync.dma_start(y_d[t, nt * 128:(nt + 1) * 128, :], st[:])
    nc.compile()
    return nc


def NIDX_R16(KCH):
    return NWIN * KCH * 128 // 16


def kernel(**inputs):
    shared, per_core, meta = _preprocess(inputs)
    shapes = {k: list(v.shape) for k, v in {**shared, **per_core[0]}.items()}
    nc = bacc.Bacc("TRN2", target_bir_lowering=False, debug=False, num_devices=NC)
    nc = _build(nc, meta, shapes)
    in_maps = [{**shared, **per_core[c]} for c in range(NC)]
    res = run_bass_kernel_spmd(nc, in_maps, core_ids=list(range(NC)))
    y = np.concatenate([res.results[c]["y"][:, :NL, :] for c in range(NC)], axis=1)
    return y.astype(np.float32)


if __name__ == "__main__":
    import reference
    inputs = {k: np.asarray(v) for k, v in reference.setup_inputs().items()}
    out = kernel(**inputs)
    exp = np.asarray(reference.reference(**inputs))
    err = np.abs(out - exp).max() / np.abs(exp).max()
    print("Relative error:", err)
